# revision 13
# baseline (speedup 1.0000x reference)
"""Bass/Tile Trainium2 kernel for the additive-attention (Bahdanau-style) module.

Computation (see problem statement):
    enc       : [src_len=2048, bs=32, enc_feat=1024]
    dec       : [bs=32, dec_hid=1024]
    W_attn    : [1024, 2048]  (W_e = [:, :1024], W_d = [:, 1024:])
    energy    = tanh(enc @ W_e.T + dec @ W_d.T + b_attn)   # [bs, src, 1024]
    scores    = energy @ w_comb                             # [bs, src]
    out       = softmax(scores, axis=src)

Sharding: data-parallel over batch — each of the 8 cores handles 4 batches.
Weights replicated. Host-side prep is layout-only (transposes / reshapes);
all FLOPs run on device.

Per-core device kernel layout choices:
  - energy tiles computed as [d_chunk=128 (partitions), n=512 (src posns)]
    so the tanh bias (dec_proj + b_attn, constant along src) is a
    per-partition ACT bias, fused into a single Tanh activation, and the
    w_comb reduction over d becomes a PE matmul accumulated over 8 d-chunks.
  - encoder tensor is staged host-side as [b][e, s] so e lands on partitions
    with fully-contiguous DMA lines (2KB runs).
  - matmuls use the float32r dtype (full-rate fp32 path on trn2: 1 cyc/row
    for moving dim >= 256 vs 4 cyc/row for plain fp32).
"""

import sys
import types

import numpy as np

# ---------------- problem constants (hardcoded per contract) ----------------
SRC_LEN = 2048
BS = 32
ENC_FEAT = 1024  # 2 * enc_hid
DEC_HID = 1024
N_CORES = 8
BPC = BS // N_CORES          # batches per core = 4
P = 128                      # partitions
EC = ENC_FEAT // P           # e-chunks = 8
DC = DEC_HID // P            # d-chunks = 8
NTILE = 512                  # src positions per matmul (fp32 moving-dim cap)
NT = SRC_LEN // NTILE        # 4 n-tiles per batch
NHALF = NT // 2              # process n-tiles in pairs (weight reuse)

MM_DTYPE = "f16"   # "f16" (FWL, device-side convert) or "f32r"
N_WARMUP = 24      # dummy PE matmuls to pre-warm the HAM clock gate

_CACHED = {}


def _install_ntff_hook_shim():
    """The agent image's antenv lacks axon_hooks; shim it so
    run_bass_kernel_spmd(trace=True) can NTFF-profile. Harmless if unused."""
    try:
        import antenv.axon_hooks  # noqa: F401
        return
    except ImportError:
        pass
    try:
        from trn_agent_boot.trn_boot import _ntff_profile_via_ctypes
        hook = _ntff_profile_via_ctypes("/opt/axon/libaxon_pjrt.so")
    except Exception:
        hook = None
    mod = types.ModuleType("antenv.axon_hooks")
    mod.get_axon_ntff_profile_hook = lambda: hook
    sys.modules["antenv.axon_hooks"] = mod


def _split_multi_waits(nc):
    """walrus in this container caps every instruction at ONE sync wait.
    Hoist extra waits onto nofuse NOPs inserted immediately before the
    instruction on the SAME engine: per-engine streams execute in order, so
    the chain preserves AND-wait semantics."""
    from concourse import mybir

    for f in nc.m.functions:
        for blk in f.blocks:
            insts = list(blk.instructions)
            out = []
            changed = False
            for inst in insts:
                si = inst.sync_info
                waits = list(si.on_wait) if si is not None and si.on_wait else []
                if len(waits) > 1:
                    changed = True
                    for k, w in enumerate(waits[:-1]):
                        n = mybir.InstNoOp(
                            name=f"{inst.name}-wsplit{k}", ins=[], outs=[]
                        )
                        n.engine = inst.engine
                        n.sync_info = mybir.SyncInfo(on_wait=[w], on_update=[])
                        out.append(n)
                    inst.sync_info = mybir.SyncInfo(
                        on_wait=[waits[-1]],
                        on_update=list(si.on_update) if si.on_update else [],
                    )
                out.append(inst)
            if changed:
                blk.instructions = out


def _patch_tile_drain():
    """The stock TileContext final drain carries one wait per logical proc
    (over the walrus 1-wait cap). Split them across chained single-wait nops
    on the sync queue, then run the generic multi-wait splitter over the
    whole module."""
    import concourse.tile as tile
    from concourse import mybir
    from concourse.vector_clock import ScopedClock

    if getattr(tile.TileContext, "_drain_split_patched", False):
        return

    def _drain_and_barrier(self, tick_clock, wait_clock):
        nc = self.nc
        probe = nc.sync.nop(nofuse=True)
        wait_clock.add_sem_waits(
            probe.ins, ScopedClock({None: tick_clock.global_clock})
        )
        si = probe.ins.sync_info
        waits = list(si.on_wait) if si is not None else []
        probe.ins.sync_info = mybir.SyncInfo(
            on_wait=waits[:1], on_update=[]
        )
        for w in waits[1:]:
            n = nc.sync.nop(nofuse=True)
            n.ins.sync_info = mybir.SyncInfo(on_wait=[w], on_update=[])
        nc.sync.drain()
        nc.all_engine_barrier()
        assert self.sems is not None
        popped = nc._tile_sem_poison_stack.pop()
        assert popped is self._sem_poison
        nc.clear_and_free_semaphores(list(self.sems.allocated().values()))
        nc.all_engine_barrier()
        _split_multi_waits(nc)

    tile.TileContext._drain_and_barrier = _drain_and_barrier
    tile.TileContext._drain_split_patched = True


def _patch_ldw_opt():
    """The default walrus invocation passes --enable-ldw-opt=false; enabling
    it lets walrus dedup back-to-back LDWEIGHTS with identical weights (our
    A/B moving-tile pairs reuse each stationary tile)."""
    import concourse.bass_utils as bu

    if getattr(bu, "_ldw_opt_patched", False):
        return
    orig = bu.bir_verify_and_optimise

    def patched(*args, **kwargs):
        import unittest.mock as um
        real_run = bu.run_command

        def run_with_flag(cmd, **kw):
            cmd = [c.replace("--enable-ldw-opt=false", "--enable-ldw-opt=true")
                   if isinstance(c, str) else c for c in cmd]
            return real_run(cmd, **kw)

        with um.patch.object(bu, "run_command", run_with_flag):
            return orig(*args, **kwargs)

    bu.bir_verify_and_optimise = patched
    # bass2jax imports the symbol lazily via concourse.bass_utils, but check:
    import concourse.bass2jax as b2j
    if hasattr(b2j, "bir_verify_and_optimise"):
        b2j.bir_verify_and_optimise = patched
    bu._ldw_opt_patched = True


def _build_nc():
    import concourse.bass as bass
    import concourse.tile as tile
    from concourse import mybir

    _patch_tile_drain()

    f32 = mybir.dt.float32
    f32r = mybir.dt.float32r
    f16 = mybir.dt.float16
    mmdt = f16 if MM_DTYPE == "f16" else f32r
    Tanh = mybir.ActivationFunctionType.Tanh
    Exp = mybir.ActivationFunctionType.Exp
    AX = mybir.AxisListType.X

    nc = bass.Bass("TRN2", target_bir_lowering=False, debug=False)
    # cache-busting marker: walrus flags aren't in the NEFF cache key
    nc.sync.nop(hint="ldwopt-v1", nofuse=True)

    enc_dt = f32r if MM_DTYPE == "f32r" else f32
    enc_t = nc.dram_tensor("enc_t", [BPC, ENC_FEAT, SRC_LEN], enc_dt,
                           kind="ExternalInput").ap()
    w_eT = nc.dram_tensor("w_eT", [ENC_FEAT, DEC_HID], enc_dt,
                          kind="ExternalInput").ap()
    w_dT = nc.dram_tensor("w_dT", [DEC_HID, DEC_HID], f32,
                          kind="ExternalInput").ap()
    dec_t = nc.dram_tensor("dec_t", [DEC_HID, BPC], f32,
                           kind="ExternalInput").ap()
    b_col = nc.dram_tensor("b_col", [P, DC], f32, kind="ExternalInput").ap()
    wc_col = nc.dram_tensor("wc_col", [P, DC], enc_dt, kind="ExternalInput").ap()
    probs = nc.dram_tensor("probs", [BPC, SRC_LEN], f32,
                           kind="ExternalOutput").ap()

    with tile.TileContext(nc) as tc:
        with (
            tc.tile_pool(name="wpool", bufs=1) as wpool,
            tc.tile_pool(name="stgp", bufs=3) as stgp,
            tc.tile_pool(name="encp", bufs=2) as encp,
            tc.tile_pool(name="actp", bufs=6) as actp,
            tc.tile_pool(name="scp", bufs=2) as scp,
            tc.tile_pool(name="smp", bufs=2) as smp,
            tc.tile_pool(name="pse", bufs=4, space="PSUM") as pse,
            tc.tile_pool(name="pss", bufs=2, space="PSUM") as pss,
            tc.tile_pool(name="psd", bufs=1, space="PSUM") as psd,
        ):
            cvt = MM_DTYPE == "f16"

            # ---- startup DMAs, priority order: wd+dec (dec_proj first on the
            # PE stream), then we, then the first enc tile ----
            wd_sb = wpool.tile([P, DC * DEC_HID], f32, tag="wd")
            for jc in range(DC):
                nc.sync.dma_start(
                    wd_sb[:, jc * DEC_HID:(jc + 1) * DEC_HID],
                    w_dT[jc * P:(jc + 1) * P, :],
                )
            dec_sb = wpool.tile([P, DC * BPC], f32, tag="dec")
            nc.sync.dma_start(dec_sb[:], dec_t.rearrange("(c p) b -> p c b", p=P))
            b_sb = wpool.tile([P, DC], f32, tag="bcol")
            nc.sync.dma_start(b_sb[:], b_col[:, :])

            wc_sb = wpool.tile([P, DC], mmdt, tag="wccol")
            if cvt:
                wc_stg = wpool.tile([P, DC], f32, tag="wcstg")
                nc.sync.dma_start(wc_stg[:], wc_col[:, :])
                nc.vector.tensor_copy(wc_sb[:], wc_stg[:])
            else:
                nc.sync.dma_start(wc_sb[:], wc_col[:, :])

            we_sb = wpool.tile([P, EC * DEC_HID], mmdt, tag="we")
            for ec in range(EC):
                dst = we_sb[:, ec * DEC_HID:(ec + 1) * DEC_HID]
                src = w_eT[ec * P:(ec + 1) * P, :]
                if cvt:
                    stg = stgp.tile([P, DEC_HID], f32, tag="stg")
                    nc.sync.dma_start(stg[:], src)
                    nc.vector.tensor_copy(dst, stg[:])
                else:
                    nc.sync.dma_start(dst, src)

            def load_enc(b, h):
                t = encp.tile([P, EC, 2 * NTILE], mmdt, tag="enc")
                base = 2 * h * NTILE
                src = enc_t[b].rearrange("(c p) s -> p c s", p=P)
                for ec in range(EC):
                    if cvt:
                        stg = stgp.tile([P, 2 * NTILE], f32, tag="stg")
                        nc.sync.dma_start(stg[:], src[:, ec, base:base + 2 * NTILE])
                        nc.vector.tensor_copy(t[:, ec, :], stg[:])
                    else:
                        nc.sync.dma_start(
                            t[:, ec, :], src[:, ec, base:base + 2 * NTILE]
                        )
                return t

            enc_first = load_enc(0, 0)

            # ---- dec_proj + b_attn -> per-(d_chunk, batch) tanh bias ----
            bias_sb = wpool.tile([P, DC * BPC], f32, tag="bias")
            for dc in range(DC):
                ps = psd.tile([P, BPC], f32, tag="dp")
                for jc in range(DC):
                    nc.tensor.matmul(
                        ps[:],
                        lhsT=wd_sb[:, jc * DEC_HID + dc * P:
                                   jc * DEC_HID + (dc + 1) * P],
                        rhs=dec_sb[:, jc * BPC:(jc + 1) * BPC],
                        start=(jc == 0),
                        stop=(jc == DC - 1),
                    )
                nc.vector.tensor_scalar_add(
                    bias_sb[:, dc * BPC:(dc + 1) * BPC], ps[:],
                    b_sb[:, dc:dc + 1],
                )

            # ---- PE warmup: dummy matmuls fill the HAM activity window while
            # the big startup DMAs stream, so real matmuls start at K=8/8 ----
            if N_WARMUP:
                wsrc = wpool.tile([P, NTILE], mmdt, tag="warm")
                nc.vector.memset(wsrc[:], 0.0)
                wps = psd.tile([P, NTILE], f32, tag="warmps")
                for _ in range(N_WARMUP):
                    nc.tensor.matmul(wps[:], lhsT=wsrc[:, 0:P], rhs=wsrc[:],
                                     start=True, stop=True)

            # ---- main loop: energy -> tanh(+bias) -> w_comb reduce ----
            # scores matmuls lag the energy groups by one d-chunk so the PE
            # never stalls on the tanh/bias chain.
            for b in range(BPC):
                sc_line = scp.tile([1, SRC_LEN], f32, tag="sc")
                for h in range(NHALF):
                    base = 2 * h * NTILE
                    enc_tile = enc_first if (b == 0 and h == 0) else load_enc(b, h)
                    ps_sA = pss.tile([1, NTILE], f32, tag="ps_s")
                    ps_sB = pss.tile([1, NTILE], f32, tag="ps_s")
                    pending = None
                    for dc in range(DC):
                        psA = pse.tile([P, NTILE], f32, tag="ps_e")
                        psB = pse.tile([P, NTILE], f32, tag="ps_e")
                        for ec in range(EC):
                            w_ap = we_sb[:, ec * DEC_HID + dc * P:
                                         ec * DEC_HID + (dc + 1) * P]
                            nc.tensor.matmul(
                                psA[:], lhsT=w_ap,
                                rhs=enc_tile[:, ec, 0:NTILE],
                                start=(ec == 0), stop=(ec == EC - 1),
                            )
                            nc.tensor.matmul(
                                psB[:], lhsT=w_ap,
                                rhs=enc_tile[:, ec, NTILE:2 * NTILE],
                                start=(ec == 0), stop=(ec == EC - 1),
                            )
                        bias_ap = bias_sb[:, dc * BPC + b:dc * BPC + b + 1]
                        thA = actp.tile([P, NTILE], mmdt, tag="th")
                        nc.scalar.activation(thA[:], psA[:], Tanh, bias=bias_ap)
                        thB = actp.tile([P, NTILE], mmdt, tag="th")
                        nc.scalar.activation(thB[:], psB[:], Tanh, bias=bias_ap)
                        if pending is not None:
                            pA, pB, pdc = pending
                            wc_ap = wc_sb[:, pdc:pdc + 1]
                            nc.tensor.matmul(
                                ps_sA[:], lhsT=wc_ap, rhs=pA[:],
                                start=(pdc == 0), stop=False,
                            )
                            nc.tensor.matmul(
                                ps_sB[:], lhsT=wc_ap, rhs=pB[:],
                                start=(pdc == 0), stop=False,
                            )
                        pending = (thA, thB, dc)
                    pA, pB, pdc = pending
                    wc_ap = wc_sb[:, pdc:pdc + 1]
                    nc.tensor.matmul(
                        ps_sA[:], lhsT=wc_ap, rhs=pA[:],
                        start=False, stop=True,
                    )
                    nc.tensor.matmul(
                        ps_sB[:], lhsT=wc_ap, rhs=pB[:],
                        start=False, stop=True,
                    )
                    nc.vector.tensor_copy(
                        sc_line[:, base:base + NTILE], ps_sA[:]
                    )
                    nc.vector.tensor_copy(
                        sc_line[:, base + NTILE:base + 2 * NTILE], ps_sB[:]
                    )

                # ---- per-batch softmax (overlaps later batches' compute) ----
                neg_mx = smp.tile([1, 1], f32, tag="negmx")
                nc.vector.reduce_max(neg_mx[:], sc_line[:], axis=AX, negate=True)
                ssum = smp.tile([1, 1], f32, tag="ssum")
                ex = smp.tile([1, SRC_LEN], f32, tag="ex")
                nc.scalar.activation(ex[:], sc_line[:], Exp,
                                     bias=neg_mx[:, 0:1], accum_out=ssum[:])
                rec = smp.tile([1, 1], f32, tag="rec")
                nc.vector.reciprocal(rec[:], ssum[:])
                nc.vector.tensor_scalar_mul(ex[:], ex[:], rec[:, 0:1])
                nc.sync.dma_start(probs[b:b + 1, :], ex[0:1, :])

    return nc


def _get_nc():
    if "nc" not in _CACHED:
        _install_ntff_hook_shim()
        _CACHED["nc"] = _build_nc()
    return _CACHED["nc"]


def _prep_in_maps(decoder_state, encoder_annotation_seq, W_attn, b_attn, w_comb):
    dec = np.asarray(decoder_state, np.float32)
    enc = np.asarray(encoder_annotation_seq, np.float32)
    W = np.asarray(W_attn, np.float32)
    ba = np.asarray(b_attn, np.float32)
    wc = np.asarray(w_comb, np.float32)

    # layout-only host prep (no FLOPs)
    encT = np.ascontiguousarray(enc.transpose(1, 2, 0))      # [bs, e, s]
    w_eT = np.ascontiguousarray(W[:, :ENC_FEAT].T)           # [e, d]
    w_dT = np.ascontiguousarray(W[:, ENC_FEAT:].T)           # [j, d]
    decT = np.ascontiguousarray(dec.T)                       # [j, bs]
    b_col = np.ascontiguousarray(ba.reshape(DC, P).T)        # [128, 8]
    wc_col = np.ascontiguousarray(wc.reshape(DC, P).T)       # [128, 8]

    in_maps = []
    for c in range(N_CORES):
        sl = slice(c * BPC, (c + 1) * BPC)
        in_maps.append({
            "enc_t": np.ascontiguousarray(encT[sl]),
            "w_eT": w_eT,
            "w_dT": w_dT,
            "dec_t": np.ascontiguousarray(decT[:, sl]),
            "b_col": b_col,
            "wc_col": wc_col,
        })
    return in_maps


def run(inputs: dict, trace: bool = False):
    """Run the SPMD kernel. Returns (full_output [32, 2048], BassKernelResults)."""
    from concourse.bass_utils import run_bass_kernel_spmd

    nc = _get_nc()
    in_maps = _prep_in_maps(**inputs)
    res = run_bass_kernel_spmd(
        nc, in_maps, core_ids=list(range(N_CORES)), trace=trace
    )
    out = np.concatenate(
        [res.results[c]["probs"] for c in range(N_CORES)], axis=0
    ).astype(np.float32)
    return out, res


def kernel(decoder_state, encoder_annotation_seq, W_attn, b_attn, w_comb):
    out, _ = run(dict(
        decoder_state=decoder_state,
        encoder_annotation_seq=encoder_annotation_seq,
        W_attn=W_attn,
        b_attn=b_attn,
        w_comb=w_comb,
    ))
    return out


# revision 15
# speedup vs baseline: 1.0303x; 1.0303x over previous
"""Bass/Tile Trainium2 kernel for the additive-attention (Bahdanau-style) module.

Computation (see problem statement):
    enc       : [src_len=2048, bs=32, enc_feat=1024]
    dec       : [bs=32, dec_hid=1024]
    W_attn    : [1024, 2048]  (W_e = [:, :1024], W_d = [:, 1024:])
    energy    = tanh(enc @ W_e.T + dec @ W_d.T + b_attn)   # [bs, src, 1024]
    scores    = energy @ w_comb                             # [bs, src]
    out       = softmax(scores, axis=src)

Sharding: data-parallel over batch — each of the 8 cores handles 4 batches.
Weights replicated. Host-side prep is layout-only (transposes / reshapes);
all FLOPs run on device.

Per-core device kernel layout choices:
  - energy tiles computed as [d_chunk=128 (partitions), n=512 (src posns)]
    so the tanh bias (dec_proj + b_attn, constant along src) is a
    per-partition ACT bias, fused into a single Tanh activation, and the
    w_comb reduction over d becomes a PE matmul accumulated over 8 d-chunks.
  - encoder tensor is staged host-side as [b][e, s] so e lands on partitions
    with fully-contiguous DMA lines (2KB runs).
  - matmuls use the float32r dtype (full-rate fp32 path on trn2: 1 cyc/row
    for moving dim >= 256 vs 4 cyc/row for plain fp32).
"""

import sys
import types

import numpy as np

# ---------------- problem constants (hardcoded per contract) ----------------
SRC_LEN = 2048
BS = 32
ENC_FEAT = 1024  # 2 * enc_hid
DEC_HID = 1024
N_CORES = 8
BPC = BS // N_CORES          # batches per core = 4
P = 128                      # partitions
EC = ENC_FEAT // P           # e-chunks = 8
DC = DEC_HID // P            # d-chunks = 8
NTILE = 512                  # src positions per matmul (fp32 moving-dim cap)
NT = SRC_LEN // NTILE        # 4 n-tiles per batch
NHALF = NT // 2              # process n-tiles in pairs (weight reuse)

MM_DTYPE = "f32r"   # "f16" (FWL, device-side convert) or "f32r"
N_WARMUP = 24      # dummy PE matmuls to pre-warm the HAM clock gate

_CACHED = {}


def _install_ntff_hook_shim():
    """The agent image's antenv lacks axon_hooks; shim it so
    run_bass_kernel_spmd(trace=True) can NTFF-profile. Harmless if unused."""
    try:
        import antenv.axon_hooks  # noqa: F401
        return
    except ImportError:
        pass
    try:
        from trn_agent_boot.trn_boot import _ntff_profile_via_ctypes
        hook = _ntff_profile_via_ctypes("/opt/axon/libaxon_pjrt.so")
    except Exception:
        hook = None
    mod = types.ModuleType("antenv.axon_hooks")
    mod.get_axon_ntff_profile_hook = lambda: hook
    sys.modules["antenv.axon_hooks"] = mod


def _split_multi_waits(nc):
    """walrus in this container caps every instruction at ONE sync wait.
    Hoist extra waits onto nofuse NOPs inserted immediately before the
    instruction on the SAME engine: per-engine streams execute in order, so
    the chain preserves AND-wait semantics."""
    from concourse import mybir

    for f in nc.m.functions:
        for blk in f.blocks:
            insts = list(blk.instructions)
            out = []
            changed = False
            for inst in insts:
                si = inst.sync_info
                waits = list(si.on_wait) if si is not None and si.on_wait else []
                if len(waits) > 1:
                    changed = True
                    for k, w in enumerate(waits[:-1]):
                        n = mybir.InstNoOp(
                            name=f"{inst.name}-wsplit{k}", ins=[], outs=[]
                        )
                        n.engine = inst.engine
                        n.sync_info = mybir.SyncInfo(on_wait=[w], on_update=[])
                        out.append(n)
                    inst.sync_info = mybir.SyncInfo(
                        on_wait=[waits[-1]],
                        on_update=list(si.on_update) if si.on_update else [],
                    )
                out.append(inst)
            if changed:
                blk.instructions = out


def _patch_tile_drain():
    """The stock TileContext final drain carries one wait per logical proc
    (over the walrus 1-wait cap). Split them across chained single-wait nops
    on the sync queue, then run the generic multi-wait splitter over the
    whole module."""
    import concourse.tile as tile
    from concourse import mybir
    from concourse.vector_clock import ScopedClock

    if getattr(tile.TileContext, "_drain_split_patched", False):
        return

    def _drain_and_barrier(self, tick_clock, wait_clock):
        nc = self.nc
        probe = nc.sync.nop(nofuse=True)
        wait_clock.add_sem_waits(
            probe.ins, ScopedClock({None: tick_clock.global_clock})
        )
        si = probe.ins.sync_info
        waits = list(si.on_wait) if si is not None else []
        probe.ins.sync_info = mybir.SyncInfo(
            on_wait=waits[:1], on_update=[]
        )
        for w in waits[1:]:
            n = nc.sync.nop(nofuse=True)
            n.ins.sync_info = mybir.SyncInfo(on_wait=[w], on_update=[])
        nc.sync.drain()
        nc.all_engine_barrier()
        assert self.sems is not None
        popped = nc._tile_sem_poison_stack.pop()
        assert popped is self._sem_poison
        nc.clear_and_free_semaphores(list(self.sems.allocated().values()))
        nc.all_engine_barrier()
        _split_multi_waits(nc)

    tile.TileContext._drain_and_barrier = _drain_and_barrier
    tile.TileContext._drain_split_patched = True


def _patch_ldw_opt():
    """The default walrus invocation passes --enable-ldw-opt=false; enabling
    it lets walrus dedup back-to-back LDWEIGHTS with identical weights (our
    A/B moving-tile pairs reuse each stationary tile)."""
    import concourse.bass_utils as bu

    if getattr(bu, "_ldw_opt_patched", False):
        return
    orig = bu.bir_verify_and_optimise

    def patched(*args, **kwargs):
        import unittest.mock as um
        real_run = bu.run_command

        def run_with_flag(cmd, **kw):
            cmd = [c.replace("--enable-ldw-opt=false", "--enable-ldw-opt=true")
                   if isinstance(c, str) else c for c in cmd]
            return real_run(cmd, **kw)

        with um.patch.object(bu, "run_command", run_with_flag):
            return orig(*args, **kwargs)

    bu.bir_verify_and_optimise = patched
    # bass2jax imports the symbol lazily via concourse.bass_utils, but check:
    import concourse.bass2jax as b2j
    if hasattr(b2j, "bir_verify_and_optimise"):
        b2j.bir_verify_and_optimise = patched
    bu._ldw_opt_patched = True


def _build_nc():
    import concourse.bass as bass
    import concourse.tile as tile
    from concourse import mybir

    _patch_tile_drain()

    f32 = mybir.dt.float32
    f32r = mybir.dt.float32r
    f16 = mybir.dt.float16
    mmdt = f16 if MM_DTYPE == "f16" else f32r
    Tanh = mybir.ActivationFunctionType.Tanh
    Exp = mybir.ActivationFunctionType.Exp
    AX = mybir.AxisListType.X

    nc = bass.Bass("TRN2", target_bir_lowering=False, debug=False)
    # cache-busting marker: walrus flags aren't in the NEFF cache key
    nc.sync.nop(hint="ldwopt-v1", nofuse=True)

    enc_dt = f32r if MM_DTYPE == "f32r" else f32
    enc_t = nc.dram_tensor("enc_t", [BPC, ENC_FEAT, SRC_LEN], enc_dt,
                           kind="ExternalInput").ap()
    w_eT = nc.dram_tensor("w_eT", [ENC_FEAT, DEC_HID], enc_dt,
                          kind="ExternalInput").ap()
    w_dT = nc.dram_tensor("w_dT", [DEC_HID, DEC_HID], f32,
                          kind="ExternalInput").ap()
    dec_t = nc.dram_tensor("dec_t", [DEC_HID, BPC], f32,
                           kind="ExternalInput").ap()
    b_col = nc.dram_tensor("b_col", [P, DC], f32, kind="ExternalInput").ap()
    wc_col = nc.dram_tensor("wc_col", [P, DC], enc_dt, kind="ExternalInput").ap()
    probs = nc.dram_tensor("probs", [BPC, SRC_LEN], f32,
                           kind="ExternalOutput").ap()

    with tile.TileContext(nc) as tc:
        with (
            tc.tile_pool(name="wpool", bufs=1) as wpool,
            tc.tile_pool(name="stgp", bufs=3) as stgp,
            tc.tile_pool(name="encp", bufs=2) as encp,
            tc.tile_pool(name="actp", bufs=6) as actp,
            tc.tile_pool(name="scp", bufs=2) as scp,
            tc.tile_pool(name="smp", bufs=2) as smp,
            tc.tile_pool(name="pse", bufs=4, space="PSUM") as pse,
            tc.tile_pool(name="pss", bufs=2, space="PSUM") as pss,
            tc.tile_pool(name="psd", bufs=1, space="PSUM") as psd,
        ):
            cvt = MM_DTYPE == "f16"

            # ---- startup DMAs, priority order: wd+dec (dec_proj first on the
            # PE stream), then we, then the first enc tile ----
            wd_sb = wpool.tile([P, DC * DEC_HID], f32, tag="wd")
            for jc in range(DC):
                nc.sync.dma_start(
                    wd_sb[:, jc * DEC_HID:(jc + 1) * DEC_HID],
                    w_dT[jc * P:(jc + 1) * P, :],
                )
            dec_sb = wpool.tile([P, DC * BPC], f32, tag="dec")
            nc.sync.dma_start(dec_sb[:], dec_t.rearrange("(c p) b -> p c b", p=P))
            b_sb = wpool.tile([P, DC], f32, tag="bcol")
            nc.sync.dma_start(b_sb[:], b_col[:, :])

            wc_sb = wpool.tile([P, DC], mmdt, tag="wccol")
            if cvt:
                wc_stg = wpool.tile([P, DC], f32, tag="wcstg")
                nc.sync.dma_start(wc_stg[:], wc_col[:, :])
                nc.vector.tensor_copy(wc_sb[:], wc_stg[:])
            else:
                nc.sync.dma_start(wc_sb[:], wc_col[:, :])

            we_sb = wpool.tile([P, EC * DEC_HID], mmdt, tag="we")
            for ec in range(EC):
                dst = we_sb[:, ec * DEC_HID:(ec + 1) * DEC_HID]
                src = w_eT[ec * P:(ec + 1) * P, :]
                if cvt:
                    stg = stgp.tile([P, DEC_HID], f32, tag="stg")
                    nc.sync.dma_start(stg[:], src)
                    nc.vector.tensor_copy(dst, stg[:])
                else:
                    nc.sync.dma_start(dst, src)

            def load_enc(b, h):
                t = encp.tile([P, EC, 2 * NTILE], mmdt, tag="enc")
                base = 2 * h * NTILE
                src = enc_t[b].rearrange("(c p) s -> p c s", p=P)
                for ec in range(EC):
                    if cvt:
                        stg = stgp.tile([P, 2 * NTILE], f32, tag="stg")
                        nc.sync.dma_start(stg[:], src[:, ec, base:base + 2 * NTILE])
                        nc.vector.tensor_copy(t[:, ec, :], stg[:])
                    else:
                        nc.sync.dma_start(
                            t[:, ec, :], src[:, ec, base:base + 2 * NTILE]
                        )
                return t

            enc_first = load_enc(0, 0)

            # ---- dec_proj + b_attn -> per-(d_chunk, batch) tanh bias ----
            bias_sb = wpool.tile([P, DC * BPC], f32, tag="bias")
            for dc in range(DC):
                ps = psd.tile([P, BPC], f32, tag="dp")
                for jc in range(DC):
                    nc.tensor.matmul(
                        ps[:],
                        lhsT=wd_sb[:, jc * DEC_HID + dc * P:
                                   jc * DEC_HID + (dc + 1) * P],
                        rhs=dec_sb[:, jc * BPC:(jc + 1) * BPC],
                        start=(jc == 0),
                        stop=(jc == DC - 1),
                    )
                nc.vector.tensor_scalar_add(
                    bias_sb[:, dc * BPC:(dc + 1) * BPC], ps[:],
                    b_sb[:, dc:dc + 1],
                )

            # ---- PE warmup: dummy matmuls fill the HAM activity window while
            # the big startup DMAs stream, so real matmuls start at K=8/8 ----
            if N_WARMUP:
                wsrc = wpool.tile([P, NTILE], f32, tag="warm")
                nc.vector.memset(wsrc[:], 0.0)
                wps = psd.tile([P, NTILE], f32, tag="warmps")
                for _ in range(N_WARMUP):
                    nc.tensor.matmul(wps[:], lhsT=wsrc[:, 0:P], rhs=wsrc[:],
                                     start=True, stop=True)

            # ---- main loop: energy -> tanh(+bias) -> w_comb reduce ----
            # scores matmuls lag the energy groups by one d-chunk so the PE
            # never stalls on the tanh/bias chain.
            for b in range(BPC):
                sc_line = scp.tile([1, SRC_LEN], f32, tag="sc")
                for h in range(NHALF):
                    base = 2 * h * NTILE
                    enc_tile = enc_first if (b == 0 and h == 0) else load_enc(b, h)
                    ps_sA = pss.tile([1, NTILE], f32, tag="ps_s")
                    ps_sB = pss.tile([1, NTILE], f32, tag="ps_s")
                    pending = None
                    for dc in range(DC):
                        psA = pse.tile([P, NTILE], f32, tag="ps_e")
                        psB = pse.tile([P, NTILE], f32, tag="ps_e")
                        for ec in range(EC):
                            w_ap = we_sb[:, ec * DEC_HID + dc * P:
                                         ec * DEC_HID + (dc + 1) * P]
                            nc.tensor.matmul(
                                psA[:], lhsT=w_ap,
                                rhs=enc_tile[:, ec, 0:NTILE],
                                start=(ec == 0), stop=(ec == EC - 1),
                            )
                            nc.tensor.matmul(
                                psB[:], lhsT=w_ap,
                                rhs=enc_tile[:, ec, NTILE:2 * NTILE],
                                start=(ec == 0), stop=(ec == EC - 1),
                            )
                        bias_ap = bias_sb[:, dc * BPC + b:dc * BPC + b + 1]
                        thA = actp.tile([P, NTILE], mmdt, tag="th")
                        nc.scalar.activation(thA[:], psA[:], Tanh, bias=bias_ap)
                        thB = actp.tile([P, NTILE], mmdt, tag="th")
                        nc.scalar.activation(thB[:], psB[:], Tanh, bias=bias_ap)
                        if pending is not None:
                            pA, pB, pdc = pending
                            wc_ap = wc_sb[:, pdc:pdc + 1]
                            nc.tensor.matmul(
                                ps_sA[:], lhsT=wc_ap, rhs=pA[:],
                                start=(pdc == 0), stop=False,
                            )
                            nc.tensor.matmul(
                                ps_sB[:], lhsT=wc_ap, rhs=pB[:],
                                start=(pdc == 0), stop=False,
                            )
                        pending = (thA, thB, dc)
                    pA, pB, pdc = pending
                    wc_ap = wc_sb[:, pdc:pdc + 1]
                    nc.tensor.matmul(
                        ps_sA[:], lhsT=wc_ap, rhs=pA[:],
                        start=False, stop=True,
                    )
                    nc.tensor.matmul(
                        ps_sB[:], lhsT=wc_ap, rhs=pB[:],
                        start=False, stop=True,
                    )
                    nc.vector.tensor_copy(
                        sc_line[:, base:base + NTILE], ps_sA[:]
                    )
                    nc.vector.tensor_copy(
                        sc_line[:, base + NTILE:base + 2 * NTILE], ps_sB[:]
                    )

                # ---- per-batch softmax (overlaps later batches' compute) ----
                neg_mx = smp.tile([1, 1], f32, tag="negmx")
                nc.vector.reduce_max(neg_mx[:], sc_line[:], axis=AX, negate=True)
                ssum = smp.tile([1, 1], f32, tag="ssum")
                ex = smp.tile([1, SRC_LEN], f32, tag="ex")
                nc.scalar.activation(ex[:], sc_line[:], Exp,
                                     bias=neg_mx[:, 0:1], accum_out=ssum[:])
                rec = smp.tile([1, 1], f32, tag="rec")
                nc.vector.reciprocal(rec[:], ssum[:])
                nc.vector.tensor_scalar_mul(ex[:], ex[:], rec[:, 0:1])
                nc.sync.dma_start(probs[b:b + 1, :], ex[0:1, :])

    return nc


def _get_nc():
    if "nc" not in _CACHED:
        _install_ntff_hook_shim()
        _CACHED["nc"] = _build_nc()
    return _CACHED["nc"]


def _prep_in_maps(decoder_state, encoder_annotation_seq, W_attn, b_attn, w_comb):
    dec = np.asarray(decoder_state, np.float32)
    enc = np.asarray(encoder_annotation_seq, np.float32)
    W = np.asarray(W_attn, np.float32)
    ba = np.asarray(b_attn, np.float32)
    wc = np.asarray(w_comb, np.float32)

    # layout-only host prep (no FLOPs)
    encT = np.ascontiguousarray(enc.transpose(1, 2, 0))      # [bs, e, s]
    w_eT = np.ascontiguousarray(W[:, :ENC_FEAT].T)           # [e, d]
    w_dT = np.ascontiguousarray(W[:, ENC_FEAT:].T)           # [j, d]
    decT = np.ascontiguousarray(dec.T)                       # [j, bs]
    b_col = np.ascontiguousarray(ba.reshape(DC, P).T)        # [128, 8]
    wc_col = np.ascontiguousarray(wc.reshape(DC, P).T)       # [128, 8]

    in_maps = []
    for c in range(N_CORES):
        sl = slice(c * BPC, (c + 1) * BPC)
        in_maps.append({
            "enc_t": np.ascontiguousarray(encT[sl]),
            "w_eT": w_eT,
            "w_dT": w_dT,
            "dec_t": np.ascontiguousarray(decT[:, sl]),
            "b_col": b_col,
            "wc_col": wc_col,
        })
    return in_maps


def run(inputs: dict, trace: bool = False):
    """Run the SPMD kernel. Returns (full_output [32, 2048], BassKernelResults)."""
    from concourse.bass_utils import run_bass_kernel_spmd

    nc = _get_nc()
    in_maps = _prep_in_maps(**inputs)
    res = run_bass_kernel_spmd(
        nc, in_maps, core_ids=list(range(N_CORES)), trace=trace
    )
    out = np.concatenate(
        [res.results[c]["probs"] for c in range(N_CORES)], axis=0
    ).astype(np.float32)
    return out, res


def kernel(decoder_state, encoder_annotation_seq, W_attn, b_attn, w_comb):
    out, _ = run(dict(
        decoder_state=decoder_state,
        encoder_annotation_seq=encoder_annotation_seq,
        W_attn=W_attn,
        b_attn=b_attn,
        w_comb=w_comb,
    ))
    return out


# revision 16
# speedup vs baseline: 1.0321x; 1.0018x over previous
"""Bass/Tile Trainium2 kernel for the additive-attention (Bahdanau-style) module.

Computation (see problem statement):
    enc       : [src_len=2048, bs=32, enc_feat=1024]
    dec       : [bs=32, dec_hid=1024]
    W_attn    : [1024, 2048]  (W_e = [:, :1024], W_d = [:, 1024:])
    energy    = tanh(enc @ W_e.T + dec @ W_d.T + b_attn)   # [bs, src, 1024]
    scores    = energy @ w_comb                             # [bs, src]
    out       = softmax(scores, axis=src)

Sharding: data-parallel over batch — each of the 8 cores handles 4 batches.
Weights replicated. Host-side prep is layout-only (transposes / reshapes);
all FLOPs run on device.

Per-core device kernel layout choices:
  - energy tiles computed as [d_chunk=128 (partitions), n=512 (src posns)]
    so the tanh bias (dec_proj + b_attn, constant along src) is a
    per-partition ACT bias, fused into a single Tanh activation, and the
    w_comb reduction over d becomes a PE matmul accumulated over 8 d-chunks.
  - encoder tensor is staged host-side as [b][e, s] so e lands on partitions
    with fully-contiguous DMA lines (2KB runs).
  - matmuls use the float32r dtype (full-rate fp32 path on trn2: 1 cyc/row
    for moving dim >= 256 vs 4 cyc/row for plain fp32).
"""

import sys
import types

import numpy as np

# ---------------- problem constants (hardcoded per contract) ----------------
SRC_LEN = 2048
BS = 32
ENC_FEAT = 1024  # 2 * enc_hid
DEC_HID = 1024
N_CORES = 8
BPC = BS // N_CORES          # batches per core = 4
P = 128                      # partitions
EC = ENC_FEAT // P           # e-chunks = 8
DC = DEC_HID // P            # d-chunks = 8
NTILE = 512                  # src positions per matmul (fp32 moving-dim cap)
NT = SRC_LEN // NTILE        # 4 n-tiles per batch
NHALF = NT // 2              # process n-tiles in pairs (weight reuse)

MM_DTYPE = "f32r"   # "f16" (FWL, device-side convert) or "f32r"
N_WARMUP = 6      # dummy PE matmuls to pre-warm the HAM clock gate

_CACHED = {}


def _install_ntff_hook_shim():
    """The agent image's antenv lacks axon_hooks; shim it so
    run_bass_kernel_spmd(trace=True) can NTFF-profile. Harmless if unused."""
    try:
        import antenv.axon_hooks  # noqa: F401
        return
    except ImportError:
        pass
    try:
        from trn_agent_boot.trn_boot import _ntff_profile_via_ctypes
        hook = _ntff_profile_via_ctypes("/opt/axon/libaxon_pjrt.so")
    except Exception:
        hook = None
    mod = types.ModuleType("antenv.axon_hooks")
    mod.get_axon_ntff_profile_hook = lambda: hook
    sys.modules["antenv.axon_hooks"] = mod


def _split_multi_waits(nc):
    """walrus in this container caps every instruction at ONE sync wait.
    Hoist extra waits onto nofuse NOPs inserted immediately before the
    instruction on the SAME engine: per-engine streams execute in order, so
    the chain preserves AND-wait semantics."""
    from concourse import mybir

    for f in nc.m.functions:
        for blk in f.blocks:
            insts = list(blk.instructions)
            out = []
            changed = False
            for inst in insts:
                si = inst.sync_info
                waits = list(si.on_wait) if si is not None and si.on_wait else []
                if len(waits) > 1:
                    changed = True
                    for k, w in enumerate(waits[:-1]):
                        n = mybir.InstNoOp(
                            name=f"{inst.name}-wsplit{k}", ins=[], outs=[]
                        )
                        n.engine = inst.engine
                        n.sync_info = mybir.SyncInfo(on_wait=[w], on_update=[])
                        out.append(n)
                    inst.sync_info = mybir.SyncInfo(
                        on_wait=[waits[-1]],
                        on_update=list(si.on_update) if si.on_update else [],
                    )
                out.append(inst)
            if changed:
                blk.instructions = out


def _patch_tile_drain():
    """The stock TileContext final drain carries one wait per logical proc
    (over the walrus 1-wait cap). Split them across chained single-wait nops
    on the sync queue, then run the generic multi-wait splitter over the
    whole module."""
    import concourse.tile as tile
    from concourse import mybir
    from concourse.vector_clock import ScopedClock

    if getattr(tile.TileContext, "_drain_split_patched", False):
        return

    def _drain_and_barrier(self, tick_clock, wait_clock):
        nc = self.nc
        probe = nc.sync.nop(nofuse=True)
        wait_clock.add_sem_waits(
            probe.ins, ScopedClock({None: tick_clock.global_clock})
        )
        si = probe.ins.sync_info
        waits = list(si.on_wait) if si is not None else []
        probe.ins.sync_info = mybir.SyncInfo(
            on_wait=waits[:1], on_update=[]
        )
        for w in waits[1:]:
            n = nc.sync.nop(nofuse=True)
            n.ins.sync_info = mybir.SyncInfo(on_wait=[w], on_update=[])
        nc.sync.drain()
        nc.all_engine_barrier()
        assert self.sems is not None
        popped = nc._tile_sem_poison_stack.pop()
        assert popped is self._sem_poison
        nc.clear_and_free_semaphores(list(self.sems.allocated().values()))
        nc.all_engine_barrier()
        _split_multi_waits(nc)

    tile.TileContext._drain_and_barrier = _drain_and_barrier
    tile.TileContext._drain_split_patched = True


def _patch_ldw_opt():
    """The default walrus invocation passes --enable-ldw-opt=false; enabling
    it lets walrus dedup back-to-back LDWEIGHTS with identical weights (our
    A/B moving-tile pairs reuse each stationary tile)."""
    import concourse.bass_utils as bu

    if getattr(bu, "_ldw_opt_patched", False):
        return
    orig = bu.bir_verify_and_optimise

    def patched(*args, **kwargs):
        import unittest.mock as um
        real_run = bu.run_command

        def run_with_flag(cmd, **kw):
            cmd = [c.replace("--enable-ldw-opt=false", "--enable-ldw-opt=true")
                   if isinstance(c, str) else c for c in cmd]
            return real_run(cmd, **kw)

        with um.patch.object(bu, "run_command", run_with_flag):
            return orig(*args, **kwargs)

    bu.bir_verify_and_optimise = patched
    # bass2jax imports the symbol lazily via concourse.bass_utils, but check:
    import concourse.bass2jax as b2j
    if hasattr(b2j, "bir_verify_and_optimise"):
        b2j.bir_verify_and_optimise = patched
    bu._ldw_opt_patched = True


def _build_nc():
    import concourse.bass as bass
    import concourse.tile as tile
    from concourse import mybir

    _patch_tile_drain()

    f32 = mybir.dt.float32
    f32r = mybir.dt.float32r
    f16 = mybir.dt.float16
    mmdt = f16 if MM_DTYPE == "f16" else f32r
    Tanh = mybir.ActivationFunctionType.Tanh
    Exp = mybir.ActivationFunctionType.Exp
    AX = mybir.AxisListType.X

    nc = bass.Bass("TRN2", target_bir_lowering=False, debug=False)
    # cache-busting marker: walrus flags aren't in the NEFF cache key
    nc.sync.nop(hint="ldwopt-v1", nofuse=True)

    enc_dt = f32r if MM_DTYPE == "f32r" else f32
    enc_t = nc.dram_tensor("enc_t", [BPC, ENC_FEAT, SRC_LEN], enc_dt,
                           kind="ExternalInput").ap()
    w_eT = nc.dram_tensor("w_eT", [ENC_FEAT, DEC_HID], enc_dt,
                          kind="ExternalInput").ap()
    w_dT = nc.dram_tensor("w_dT", [DEC_HID, DEC_HID], f32,
                          kind="ExternalInput").ap()
    dec_t = nc.dram_tensor("dec_t", [DEC_HID, BPC], f32,
                           kind="ExternalInput").ap()
    b_col = nc.dram_tensor("b_col", [P, DC], f32, kind="ExternalInput").ap()
    wc_col = nc.dram_tensor("wc_col", [P, DC], enc_dt, kind="ExternalInput").ap()
    probs = nc.dram_tensor("probs", [BPC, SRC_LEN], f32,
                           kind="ExternalOutput").ap()

    with tile.TileContext(nc) as tc:
        with (
            tc.tile_pool(name="wpool", bufs=1) as wpool,
            tc.tile_pool(name="stgp", bufs=3) as stgp,
            tc.tile_pool(name="encp", bufs=2) as encp,
            tc.tile_pool(name="actp", bufs=6) as actp,
            tc.tile_pool(name="scp", bufs=2) as scp,
            tc.tile_pool(name="smp", bufs=2) as smp,
            tc.tile_pool(name="pse", bufs=4, space="PSUM") as pse,
            tc.tile_pool(name="pss", bufs=2, space="PSUM") as pss,
            tc.tile_pool(name="psd", bufs=1, space="PSUM") as psd,
        ):
            cvt = MM_DTYPE == "f16"

            # ---- startup DMAs, priority order: wd+dec (dec_proj first on the
            # PE stream), then we, then the first enc tile ----
            wd_sb = wpool.tile([P, DC * DEC_HID], f32, tag="wd")
            for jc in range(DC):
                nc.sync.dma_start(
                    wd_sb[:, jc * DEC_HID:(jc + 1) * DEC_HID],
                    w_dT[jc * P:(jc + 1) * P, :],
                )
            dec_sb = wpool.tile([P, DC * BPC], f32, tag="dec")
            nc.sync.dma_start(dec_sb[:], dec_t.rearrange("(c p) b -> p c b", p=P))
            b_sb = wpool.tile([P, DC], f32, tag="bcol")
            nc.sync.dma_start(b_sb[:], b_col[:, :])

            wc_sb = wpool.tile([P, DC], mmdt, tag="wccol")
            if cvt:
                wc_stg = wpool.tile([P, DC], f32, tag="wcstg")
                nc.sync.dma_start(wc_stg[:], wc_col[:, :])
                nc.vector.tensor_copy(wc_sb[:], wc_stg[:])
            else:
                nc.sync.dma_start(wc_sb[:], wc_col[:, :])

            we_sb = wpool.tile([P, EC * DEC_HID], mmdt, tag="we")
            for ec in range(EC):
                dst = we_sb[:, ec * DEC_HID:(ec + 1) * DEC_HID]
                src = w_eT[ec * P:(ec + 1) * P, :]
                if cvt:
                    stg = stgp.tile([P, DEC_HID], f32, tag="stg")
                    nc.sync.dma_start(stg[:], src)
                    nc.vector.tensor_copy(dst, stg[:])
                else:
                    nc.sync.dma_start(dst, src)

            def load_enc(b, h):
                t = encp.tile([P, EC, 2 * NTILE], mmdt, tag="enc")
                base = 2 * h * NTILE
                src = enc_t[b].rearrange("(c p) s -> p c s", p=P)
                for ec in range(EC):
                    if cvt:
                        stg = stgp.tile([P, 2 * NTILE], f32, tag="stg")
                        nc.sync.dma_start(stg[:], src[:, ec, base:base + 2 * NTILE])
                        nc.vector.tensor_copy(t[:, ec, :], stg[:])
                    else:
                        nc.sync.dma_start(
                            t[:, ec, :], src[:, ec, base:base + 2 * NTILE]
                        )
                return t

            enc_first = load_enc(0, 0)

            # ---- dec_proj + b_attn -> per-(d_chunk, batch) tanh bias ----
            bias_sb = wpool.tile([P, DC * BPC], f32, tag="bias")
            for dc in range(DC):
                ps = psd.tile([P, BPC], f32, tag="dp")
                for jc in range(DC):
                    nc.tensor.matmul(
                        ps[:],
                        lhsT=wd_sb[:, jc * DEC_HID + dc * P:
                                   jc * DEC_HID + (dc + 1) * P],
                        rhs=dec_sb[:, jc * BPC:(jc + 1) * BPC],
                        start=(jc == 0),
                        stop=(jc == DC - 1),
                    )
                nc.vector.tensor_scalar_add(
                    bias_sb[:, dc * BPC:(dc + 1) * BPC], ps[:],
                    b_sb[:, dc:dc + 1],
                )

            # ---- PE warmup: dummy matmuls fill the HAM activity window while
            # the big startup DMAs stream, so real matmuls start at K=8/8 ----
            if N_WARMUP:
                wsrc = wpool.tile([P, NTILE], f32, tag="warm")
                nc.vector.memset(wsrc[:], 0.0)
                wps = psd.tile([P, NTILE], f32, tag="warmps")
                for _ in range(N_WARMUP):
                    nc.tensor.matmul(wps[:], lhsT=wsrc[:, 0:P], rhs=wsrc[:],
                                     start=True, stop=True)

            # ---- main loop: energy -> tanh(+bias) -> w_comb reduce ----
            # scores matmuls lag the energy groups by one d-chunk so the PE
            # never stalls on the tanh/bias chain.
            for b in range(BPC):
                sc_line = scp.tile([1, SRC_LEN], f32, tag="sc")
                for h in range(NHALF):
                    base = 2 * h * NTILE
                    enc_tile = enc_first if (b == 0 and h == 0) else load_enc(b, h)
                    ps_sA = pss.tile([1, NTILE], f32, tag="ps_s")
                    ps_sB = pss.tile([1, NTILE], f32, tag="ps_s")
                    pending = None
                    for dc in range(DC):
                        psA = pse.tile([P, NTILE], f32, tag="ps_e")
                        psB = pse.tile([P, NTILE], f32, tag="ps_e")
                        for ec in range(EC):
                            w_ap = we_sb[:, ec * DEC_HID + dc * P:
                                         ec * DEC_HID + (dc + 1) * P]
                            nc.tensor.matmul(
                                psA[:], lhsT=w_ap,
                                rhs=enc_tile[:, ec, 0:NTILE],
                                start=(ec == 0), stop=(ec == EC - 1),
                            )
                            nc.tensor.matmul(
                                psB[:], lhsT=w_ap,
                                rhs=enc_tile[:, ec, NTILE:2 * NTILE],
                                start=(ec == 0), stop=(ec == EC - 1),
                            )
                        bias_ap = bias_sb[:, dc * BPC + b:dc * BPC + b + 1]
                        thA = actp.tile([P, NTILE], mmdt, tag="th")
                        nc.scalar.activation(thA[:], psA[:], Tanh, bias=bias_ap)
                        thB = actp.tile([P, NTILE], mmdt, tag="th")
                        nc.scalar.activation(thB[:], psB[:], Tanh, bias=bias_ap)
                        if pending is not None:
                            pA, pB, pdc = pending
                            wc_ap = wc_sb[:, pdc:pdc + 1]
                            nc.tensor.matmul(
                                ps_sA[:], lhsT=wc_ap, rhs=pA[:],
                                start=(pdc == 0), stop=False,
                            )
                            nc.tensor.matmul(
                                ps_sB[:], lhsT=wc_ap, rhs=pB[:],
                                start=(pdc == 0), stop=False,
                            )
                        pending = (thA, thB, dc)
                    pA, pB, pdc = pending
                    wc_ap = wc_sb[:, pdc:pdc + 1]
                    nc.tensor.matmul(
                        ps_sA[:], lhsT=wc_ap, rhs=pA[:],
                        start=False, stop=True,
                    )
                    nc.tensor.matmul(
                        ps_sB[:], lhsT=wc_ap, rhs=pB[:],
                        start=False, stop=True,
                    )
                    nc.vector.tensor_copy(
                        sc_line[:, base:base + NTILE], ps_sA[:]
                    )
                    nc.vector.tensor_copy(
                        sc_line[:, base + NTILE:base + 2 * NTILE], ps_sB[:]
                    )

                # ---- per-batch softmax (overlaps later batches' compute) ----
                neg_mx = smp.tile([1, 1], f32, tag="negmx")
                nc.vector.reduce_max(neg_mx[:], sc_line[:], axis=AX, negate=True)
                ssum = smp.tile([1, 1], f32, tag="ssum")
                ex = smp.tile([1, SRC_LEN], f32, tag="ex")
                nc.scalar.activation(ex[:], sc_line[:], Exp,
                                     bias=neg_mx[:, 0:1], accum_out=ssum[:])
                rec = smp.tile([1, 1], f32, tag="rec")
                nc.vector.reciprocal(rec[:], ssum[:])
                nc.vector.tensor_scalar_mul(ex[:], ex[:], rec[:, 0:1])
                nc.sync.dma_start(probs[b:b + 1, :], ex[0:1, :])

    return nc


def _get_nc():
    if "nc" not in _CACHED:
        _install_ntff_hook_shim()
        _CACHED["nc"] = _build_nc()
    return _CACHED["nc"]


def _prep_in_maps(decoder_state, encoder_annotation_seq, W_attn, b_attn, w_comb):
    dec = np.asarray(decoder_state, np.float32)
    enc = np.asarray(encoder_annotation_seq, np.float32)
    W = np.asarray(W_attn, np.float32)
    ba = np.asarray(b_attn, np.float32)
    wc = np.asarray(w_comb, np.float32)

    # layout-only host prep (no FLOPs)
    encT = np.ascontiguousarray(enc.transpose(1, 2, 0))      # [bs, e, s]
    w_eT = np.ascontiguousarray(W[:, :ENC_FEAT].T)           # [e, d]
    w_dT = np.ascontiguousarray(W[:, ENC_FEAT:].T)           # [j, d]
    decT = np.ascontiguousarray(dec.T)                       # [j, bs]
    b_col = np.ascontiguousarray(ba.reshape(DC, P).T)        # [128, 8]
    wc_col = np.ascontiguousarray(wc.reshape(DC, P).T)       # [128, 8]

    in_maps = []
    for c in range(N_CORES):
        sl = slice(c * BPC, (c + 1) * BPC)
        in_maps.append({
            "enc_t": np.ascontiguousarray(encT[sl]),
            "w_eT": w_eT,
            "w_dT": w_dT,
            "dec_t": np.ascontiguousarray(decT[:, sl]),
            "b_col": b_col,
            "wc_col": wc_col,
        })
    return in_maps


def run(inputs: dict, trace: bool = False):
    """Run the SPMD kernel. Returns (full_output [32, 2048], BassKernelResults)."""
    from concourse.bass_utils import run_bass_kernel_spmd

    nc = _get_nc()
    in_maps = _prep_in_maps(**inputs)
    res = run_bass_kernel_spmd(
        nc, in_maps, core_ids=list(range(N_CORES)), trace=trace
    )
    out = np.concatenate(
        [res.results[c]["probs"] for c in range(N_CORES)], axis=0
    ).astype(np.float32)
    return out, res


def kernel(decoder_state, encoder_annotation_seq, W_attn, b_attn, w_comb):
    out, _ = run(dict(
        decoder_state=decoder_state,
        encoder_annotation_seq=encoder_annotation_seq,
        W_attn=W_attn,
        b_attn=b_attn,
        w_comb=w_comb,
    ))
    return out


# revision 17
# speedup vs baseline: 1.0400x; 1.0077x over previous
"""Bass/Tile Trainium2 kernel for the additive-attention (Bahdanau-style) module.

Computation (see problem statement):
    enc       : [src_len=2048, bs=32, enc_feat=1024]
    dec       : [bs=32, dec_hid=1024]
    W_attn    : [1024, 2048]  (W_e = [:, :1024], W_d = [:, 1024:])
    energy    = tanh(enc @ W_e.T + dec @ W_d.T + b_attn)   # [bs, src, 1024]
    scores    = energy @ w_comb                             # [bs, src]
    out       = softmax(scores, axis=src)

Sharding: data-parallel over batch — each of the 8 cores handles 4 batches.
Weights replicated. Host-side prep is layout-only (transposes / reshapes);
all FLOPs run on device.

Per-core device kernel layout choices:
  - energy tiles computed as [d_chunk=128 (partitions), n=512 (src posns)]
    so the tanh bias (dec_proj + b_attn, constant along src) is a
    per-partition ACT bias, fused into a single Tanh activation, and the
    w_comb reduction over d becomes a PE matmul accumulated over 8 d-chunks.
  - encoder tensor is staged host-side as [b][e, s] so e lands on partitions
    with fully-contiguous DMA lines (2KB runs).
  - matmuls use the float32r dtype (full-rate fp32 path on trn2: 1 cyc/row
    for moving dim >= 256 vs 4 cyc/row for plain fp32).
"""

import sys
import types

import numpy as np

# ---------------- problem constants (hardcoded per contract) ----------------
SRC_LEN = 2048
BS = 32
ENC_FEAT = 1024  # 2 * enc_hid
DEC_HID = 1024
N_CORES = 8
BPC = BS // N_CORES          # batches per core = 4
P = 128                      # partitions
EC = ENC_FEAT // P           # e-chunks = 8
DC = DEC_HID // P            # d-chunks = 8
NTILE = 512                  # src positions per matmul (fp32 moving-dim cap)
NT = SRC_LEN // NTILE        # 4 n-tiles per batch
NHALF = NT // 2              # process n-tiles in pairs (weight reuse)

MM_DTYPE = "f32r"   # "f16" (FWL, device-side convert) or "f32r"
N_WARMUP = 0      # dummy PE matmuls to pre-warm the HAM clock gate

_CACHED = {}


def _install_ntff_hook_shim():
    """The agent image's antenv lacks axon_hooks; shim it so
    run_bass_kernel_spmd(trace=True) can NTFF-profile. Harmless if unused."""
    try:
        import antenv.axon_hooks  # noqa: F401
        return
    except ImportError:
        pass
    try:
        from trn_agent_boot.trn_boot import _ntff_profile_via_ctypes
        hook = _ntff_profile_via_ctypes("/opt/axon/libaxon_pjrt.so")
    except Exception:
        hook = None
    mod = types.ModuleType("antenv.axon_hooks")
    mod.get_axon_ntff_profile_hook = lambda: hook
    sys.modules["antenv.axon_hooks"] = mod


def _split_multi_waits(nc):
    """walrus in this container caps every instruction at ONE sync wait.
    Hoist extra waits onto nofuse NOPs inserted immediately before the
    instruction on the SAME engine: per-engine streams execute in order, so
    the chain preserves AND-wait semantics."""
    from concourse import mybir

    for f in nc.m.functions:
        for blk in f.blocks:
            insts = list(blk.instructions)
            out = []
            changed = False
            for inst in insts:
                si = inst.sync_info
                waits = list(si.on_wait) if si is not None and si.on_wait else []
                if len(waits) > 1:
                    changed = True
                    for k, w in enumerate(waits[:-1]):
                        n = mybir.InstNoOp(
                            name=f"{inst.name}-wsplit{k}", ins=[], outs=[]
                        )
                        n.engine = inst.engine
                        n.sync_info = mybir.SyncInfo(on_wait=[w], on_update=[])
                        out.append(n)
                    inst.sync_info = mybir.SyncInfo(
                        on_wait=[waits[-1]],
                        on_update=list(si.on_update) if si.on_update else [],
                    )
                out.append(inst)
            if changed:
                blk.instructions = out


def _patch_tile_drain():
    """The stock TileContext final drain carries one wait per logical proc
    (over the walrus 1-wait cap). Split them across chained single-wait nops
    on the sync queue, then run the generic multi-wait splitter over the
    whole module."""
    import concourse.tile as tile
    from concourse import mybir
    from concourse.vector_clock import ScopedClock

    if getattr(tile.TileContext, "_drain_split_patched", False):
        return

    def _drain_and_barrier(self, tick_clock, wait_clock):
        nc = self.nc
        probe = nc.sync.nop(nofuse=True)
        wait_clock.add_sem_waits(
            probe.ins, ScopedClock({None: tick_clock.global_clock})
        )
        si = probe.ins.sync_info
        waits = list(si.on_wait) if si is not None else []
        probe.ins.sync_info = mybir.SyncInfo(
            on_wait=waits[:1], on_update=[]
        )
        for w in waits[1:]:
            n = nc.sync.nop(nofuse=True)
            n.ins.sync_info = mybir.SyncInfo(on_wait=[w], on_update=[])
        nc.sync.drain()
        nc.all_engine_barrier()
        assert self.sems is not None
        popped = nc._tile_sem_poison_stack.pop()
        assert popped is self._sem_poison
        nc.clear_and_free_semaphores(list(self.sems.allocated().values()))
        nc.all_engine_barrier()
        _split_multi_waits(nc)

    tile.TileContext._drain_and_barrier = _drain_and_barrier
    tile.TileContext._drain_split_patched = True


def _patch_ldw_opt():
    """The default walrus invocation passes --enable-ldw-opt=false; enabling
    it lets walrus dedup back-to-back LDWEIGHTS with identical weights (our
    A/B moving-tile pairs reuse each stationary tile)."""
    import concourse.bass_utils as bu

    if getattr(bu, "_ldw_opt_patched", False):
        return
    orig = bu.bir_verify_and_optimise

    def patched(*args, **kwargs):
        import unittest.mock as um
        real_run = bu.run_command

        def run_with_flag(cmd, **kw):
            cmd = [c.replace("--enable-ldw-opt=false", "--enable-ldw-opt=true")
                   if isinstance(c, str) else c for c in cmd]
            return real_run(cmd, **kw)

        with um.patch.object(bu, "run_command", run_with_flag):
            return orig(*args, **kwargs)

    bu.bir_verify_and_optimise = patched
    # bass2jax imports the symbol lazily via concourse.bass_utils, but check:
    import concourse.bass2jax as b2j
    if hasattr(b2j, "bir_verify_and_optimise"):
        b2j.bir_verify_and_optimise = patched
    bu._ldw_opt_patched = True


def _build_nc():
    import concourse.bass as bass
    import concourse.tile as tile
    from concourse import mybir

    _patch_tile_drain()
    if MM_DTYPE != "f16":
        _patch_ldw_opt()

    f32 = mybir.dt.float32
    f32r = mybir.dt.float32r
    f16 = mybir.dt.float16
    mmdt = f16 if MM_DTYPE == "f16" else f32r
    Tanh = mybir.ActivationFunctionType.Tanh
    Exp = mybir.ActivationFunctionType.Exp
    AX = mybir.AxisListType.X

    nc = bass.Bass("TRN2", target_bir_lowering=False, debug=False)
    # cache-busting marker: walrus flags aren't in the NEFF cache key
    nc.sync.nop(hint="ldwopt-v1", nofuse=True)

    enc_dt = f32r if MM_DTYPE == "f32r" else f32
    enc_t = nc.dram_tensor("enc_t", [BPC, ENC_FEAT, SRC_LEN], enc_dt,
                           kind="ExternalInput").ap()
    w_eT = nc.dram_tensor("w_eT", [ENC_FEAT, DEC_HID], enc_dt,
                          kind="ExternalInput").ap()
    w_dT = nc.dram_tensor("w_dT", [DEC_HID, DEC_HID], f32,
                          kind="ExternalInput").ap()
    dec_t = nc.dram_tensor("dec_t", [DEC_HID, BPC], f32,
                           kind="ExternalInput").ap()
    b_col = nc.dram_tensor("b_col", [P, DC], f32, kind="ExternalInput").ap()
    wc_col = nc.dram_tensor("wc_col", [P, DC], enc_dt, kind="ExternalInput").ap()
    probs = nc.dram_tensor("probs", [BPC, SRC_LEN], f32,
                           kind="ExternalOutput").ap()

    with tile.TileContext(nc) as tc:
        with (
            tc.tile_pool(name="wpool", bufs=1) as wpool,
            tc.tile_pool(name="stgp", bufs=3) as stgp,
            tc.tile_pool(name="encp", bufs=2) as encp,
            tc.tile_pool(name="actp", bufs=6) as actp,
            tc.tile_pool(name="scp", bufs=2) as scp,
            tc.tile_pool(name="smp", bufs=2) as smp,
            tc.tile_pool(name="pse", bufs=4, space="PSUM") as pse,
            tc.tile_pool(name="pss", bufs=2, space="PSUM") as pss,
            tc.tile_pool(name="psd", bufs=1, space="PSUM") as psd,
        ):
            cvt = MM_DTYPE == "f16"

            # ---- startup DMAs, priority order: wd+dec (dec_proj first on the
            # PE stream), then we, then the first enc tile ----
            wd_sb = wpool.tile([P, DC * DEC_HID], f32, tag="wd")
            for jc in range(DC):
                nc.sync.dma_start(
                    wd_sb[:, jc * DEC_HID:(jc + 1) * DEC_HID],
                    w_dT[jc * P:(jc + 1) * P, :],
                )
            dec_sb = wpool.tile([P, DC * BPC], f32, tag="dec")
            nc.sync.dma_start(dec_sb[:], dec_t.rearrange("(c p) b -> p c b", p=P))
            b_sb = wpool.tile([P, DC], f32, tag="bcol")
            nc.sync.dma_start(b_sb[:], b_col[:, :])

            wc_sb = wpool.tile([P, DC], mmdt, tag="wccol")
            if cvt:
                wc_stg = wpool.tile([P, DC], f32, tag="wcstg")
                nc.sync.dma_start(wc_stg[:], wc_col[:, :])
                nc.vector.tensor_copy(wc_sb[:], wc_stg[:])
            else:
                nc.sync.dma_start(wc_sb[:], wc_col[:, :])

            we_sb = wpool.tile([P, EC * DEC_HID], mmdt, tag="we")

            def load_we(ec):
                dst = we_sb[:, ec * DEC_HID:(ec + 1) * DEC_HID]
                src = w_eT[ec * P:(ec + 1) * P, :]
                if cvt:
                    stg = stgp.tile([P, DEC_HID], f32, tag="stg")
                    nc.sync.dma_start(stg[:], src)
                    nc.gpsimd.tensor_copy(dst, stg[:])
                else:
                    nc.sync.dma_start(dst, src)

            def load_enc_chunk(t, b, h, ec):
                base = 2 * h * NTILE
                src = enc_t[b].rearrange("(c p) s -> p c s", p=P)
                if cvt:
                    stg = stgp.tile([P, 2 * NTILE], f32, tag="stg")
                    nc.sync.dma_start(stg[:], src[:, ec, base:base + 2 * NTILE])
                    nc.gpsimd.tensor_copy(t[:, ec, :], stg[:])
                else:
                    nc.sync.dma_start(
                        t[:, ec, :], src[:, ec, base:base + 2 * NTILE]
                    )

            def load_enc(b, h):
                t = encp.tile([P, EC, 2 * NTILE], mmdt, tag="enc")
                for ec in range(EC):
                    load_enc_chunk(t, b, h, ec)
                return t

            # first tile: interleave we/enc chunk DMAs so energy matmuls can
            # begin as soon as the first (we, enc) chunk pair lands
            enc_first = encp.tile([P, EC, 2 * NTILE], mmdt, tag="enc")
            for ec in range(EC):
                load_we(ec)
                load_enc_chunk(enc_first, 0, 0, ec)

            # ---- dec_proj + b_attn -> per-(d_chunk, batch) tanh bias ----
            bias_sb = wpool.tile([P, DC * BPC], f32, tag="bias")
            for dc in range(DC):
                ps = psd.tile([P, BPC], f32, tag="dp")
                for jc in range(DC):
                    nc.tensor.matmul(
                        ps[:],
                        lhsT=wd_sb[:, jc * DEC_HID + dc * P:
                                   jc * DEC_HID + (dc + 1) * P],
                        rhs=dec_sb[:, jc * BPC:(jc + 1) * BPC],
                        start=(jc == 0),
                        stop=(jc == DC - 1),
                    )
                nc.vector.tensor_scalar_add(
                    bias_sb[:, dc * BPC:(dc + 1) * BPC], ps[:],
                    b_sb[:, dc:dc + 1],
                )

            # ---- PE warmup: dummy matmuls fill the HAM activity window while
            # the big startup DMAs stream, so real matmuls start at K=8/8 ----
            if N_WARMUP:
                wsrc = wpool.tile([P, NTILE], f32, tag="warm")
                nc.vector.memset(wsrc[:], 0.0)
                wps = psd.tile([P, NTILE], f32, tag="warmps")
                for _ in range(N_WARMUP):
                    nc.tensor.matmul(wps[:], lhsT=wsrc[:, 0:P], rhs=wsrc[:],
                                     start=True, stop=True)

            # ---- main loop: energy -> tanh(+bias) -> w_comb reduce ----
            # scores matmuls lag the energy groups by one d-chunk so the PE
            # never stalls on the tanh/bias chain.
            for b in range(BPC):
                sc_line = scp.tile([1, SRC_LEN], f32, tag="sc")
                for h in range(NHALF):
                    base = 2 * h * NTILE
                    enc_tile = enc_first if (b == 0 and h == 0) else load_enc(b, h)
                    ps_sA = pss.tile([1, NTILE], f32, tag="ps_s")
                    ps_sB = pss.tile([1, NTILE], f32, tag="ps_s")
                    pending = None
                    for dc in range(DC):
                        psA = pse.tile([P, NTILE], f32, tag="ps_e")
                        psB = pse.tile([P, NTILE], f32, tag="ps_e")
                        for ec in range(EC):
                            w_ap = we_sb[:, ec * DEC_HID + dc * P:
                                         ec * DEC_HID + (dc + 1) * P]
                            nc.tensor.matmul(
                                psA[:], lhsT=w_ap,
                                rhs=enc_tile[:, ec, 0:NTILE],
                                start=(ec == 0), stop=(ec == EC - 1),
                            )
                            nc.tensor.matmul(
                                psB[:], lhsT=w_ap,
                                rhs=enc_tile[:, ec, NTILE:2 * NTILE],
                                start=(ec == 0), stop=(ec == EC - 1),
                            )
                        bias_ap = bias_sb[:, dc * BPC + b:dc * BPC + b + 1]
                        thA = actp.tile([P, NTILE], mmdt, tag="th")
                        nc.scalar.activation(thA[:], psA[:], Tanh, bias=bias_ap)
                        thB = actp.tile([P, NTILE], mmdt, tag="th")
                        nc.scalar.activation(thB[:], psB[:], Tanh, bias=bias_ap)
                        if pending is not None:
                            pA, pB, pdc = pending
                            wc_ap = wc_sb[:, pdc:pdc + 1]
                            nc.tensor.matmul(
                                ps_sA[:], lhsT=wc_ap, rhs=pA[:],
                                start=(pdc == 0), stop=False,
                            )
                            nc.tensor.matmul(
                                ps_sB[:], lhsT=wc_ap, rhs=pB[:],
                                start=(pdc == 0), stop=False,
                            )
                        pending = (thA, thB, dc)
                    pA, pB, pdc = pending
                    wc_ap = wc_sb[:, pdc:pdc + 1]
                    nc.tensor.matmul(
                        ps_sA[:], lhsT=wc_ap, rhs=pA[:],
                        start=False, stop=True,
                    )
                    nc.tensor.matmul(
                        ps_sB[:], lhsT=wc_ap, rhs=pB[:],
                        start=False, stop=True,
                    )
                    nc.vector.tensor_copy(
                        sc_line[:, base:base + NTILE], ps_sA[:]
                    )
                    nc.vector.tensor_copy(
                        sc_line[:, base + NTILE:base + 2 * NTILE], ps_sB[:]
                    )

                # ---- per-batch softmax (overlaps later batches' compute) ----
                neg_mx = smp.tile([1, 1], f32, tag="negmx")
                nc.vector.reduce_max(neg_mx[:], sc_line[:], axis=AX, negate=True)
                ssum = smp.tile([1, 1], f32, tag="ssum")
                ex = smp.tile([1, SRC_LEN], f32, tag="ex")
                nc.scalar.activation(ex[:], sc_line[:], Exp,
                                     bias=neg_mx[:, 0:1], accum_out=ssum[:])
                rec = smp.tile([1, 1], f32, tag="rec")
                nc.vector.reciprocal(rec[:], ssum[:])
                nc.vector.tensor_scalar_mul(ex[:], ex[:], rec[:, 0:1])
                nc.sync.dma_start(probs[b:b + 1, :], ex[0:1, :])

    return nc


def _get_nc():
    if "nc" not in _CACHED:
        _install_ntff_hook_shim()
        _CACHED["nc"] = _build_nc()
    return _CACHED["nc"]


def _prep_in_maps(decoder_state, encoder_annotation_seq, W_attn, b_attn, w_comb):
    dec = np.asarray(decoder_state, np.float32)
    enc = np.asarray(encoder_annotation_seq, np.float32)
    W = np.asarray(W_attn, np.float32)
    ba = np.asarray(b_attn, np.float32)
    wc = np.asarray(w_comb, np.float32)

    # layout-only host prep (no FLOPs)
    encT = np.ascontiguousarray(enc.transpose(1, 2, 0))      # [bs, e, s]
    w_eT = np.ascontiguousarray(W[:, :ENC_FEAT].T)           # [e, d]
    w_dT = np.ascontiguousarray(W[:, ENC_FEAT:].T)           # [j, d]
    decT = np.ascontiguousarray(dec.T)                       # [j, bs]
    b_col = np.ascontiguousarray(ba.reshape(DC, P).T)        # [128, 8]
    wc_col = np.ascontiguousarray(wc.reshape(DC, P).T)       # [128, 8]

    in_maps = []
    for c in range(N_CORES):
        sl = slice(c * BPC, (c + 1) * BPC)
        in_maps.append({
            "enc_t": np.ascontiguousarray(encT[sl]),
            "w_eT": w_eT,
            "w_dT": w_dT,
            "dec_t": np.ascontiguousarray(decT[:, sl]),
            "b_col": b_col,
            "wc_col": wc_col,
        })
    return in_maps


def run(inputs: dict, trace: bool = False):
    """Run the SPMD kernel. Returns (full_output [32, 2048], BassKernelResults)."""
    from concourse.bass_utils import run_bass_kernel_spmd

    nc = _get_nc()
    in_maps = _prep_in_maps(**inputs)
    res = run_bass_kernel_spmd(
        nc, in_maps, core_ids=list(range(N_CORES)), trace=trace
    )
    out = np.concatenate(
        [res.results[c]["probs"] for c in range(N_CORES)], axis=0
    ).astype(np.float32)
    return out, res


def kernel(decoder_state, encoder_annotation_seq, W_attn, b_attn, w_comb):
    out, _ = run(dict(
        decoder_state=decoder_state,
        encoder_annotation_seq=encoder_annotation_seq,
        W_attn=W_attn,
        b_attn=b_attn,
        w_comb=w_comb,
    ))
    return out


# revision 23
# speedup vs baseline: 1.0950x; 1.0528x over previous
"""Bass/Tile Trainium2 kernel for the additive-attention (Bahdanau-style) module.

Computation (see problem statement):
    enc       : [src_len=2048, bs=32, enc_feat=1024]
    dec       : [bs=32, dec_hid=1024]
    W_attn    : [1024, 2048]  (W_e = [:, :1024], W_d = [:, 1024:])
    energy    = tanh(enc @ W_e.T + dec @ W_d.T + b_attn)   # [bs, src, 1024]
    scores    = energy @ w_comb                             # [bs, src]
    out       = softmax(scores, axis=src)

Sharding: data-parallel over batch — each of the 8 cores handles 4 batches.
Weights replicated. Host-side prep is layout-only (transposes / reshapes);
all FLOPs run on device.

Per-core device kernel layout choices:
  - energy tiles computed as [d_chunk=128 (partitions), n=512 (src posns)]
    so the tanh bias (dec_proj + b_attn, constant along src) is a
    per-partition ACT bias, fused into a single Tanh activation, and the
    w_comb reduction over d becomes a PE matmul accumulated over 8 d-chunks.
  - encoder tensor is staged host-side as [b][e, s] so e lands on partitions
    with fully-contiguous DMA lines (2KB runs).
  - matmuls use the float32r dtype (full-rate fp32 path on trn2: 1 cyc/row
    for moving dim >= 256 vs 4 cyc/row for plain fp32).
"""

import sys
import types

import numpy as np

# ---------------- problem constants (hardcoded per contract) ----------------
SRC_LEN = 2048
BS = 32
ENC_FEAT = 1024  # 2 * enc_hid
DEC_HID = 1024
N_CORES = 8
BPC = BS // N_CORES          # batches per core = 4
P = 128                      # partitions
EC = ENC_FEAT // P           # e-chunks = 8
DC = DEC_HID // P            # d-chunks = 8
NTILE = 512                  # src positions per matmul (fp32 moving-dim cap)
NT = SRC_LEN // NTILE        # 4 n-tiles per batch
NHALF = NT // 2              # process n-tiles in pairs (weight reuse)

MM_DTYPE = "f32r"   # "f16" (FWL, device-side convert) or "f32r"
N_WARMUP = 0      # dummy PE matmuls to pre-warm the HAM clock gate

_CACHED = {}


def _install_ntff_hook_shim():
    """The agent image's antenv lacks axon_hooks; shim it so
    run_bass_kernel_spmd(trace=True) can NTFF-profile. Harmless if unused."""
    try:
        import antenv.axon_hooks  # noqa: F401
        return
    except ImportError:
        pass
    try:
        from trn_agent_boot.trn_boot import _ntff_profile_via_ctypes
        hook = _ntff_profile_via_ctypes("/opt/axon/libaxon_pjrt.so")
    except Exception:
        hook = None
    mod = types.ModuleType("antenv.axon_hooks")
    mod.get_axon_ntff_profile_hook = lambda: hook
    sys.modules["antenv.axon_hooks"] = mod


def _split_multi_waits(nc):
    """walrus in this container caps every instruction at ONE sync wait.
    Hoist extra waits onto nofuse NOPs inserted immediately before the
    instruction on the SAME engine: per-engine streams execute in order, so
    the chain preserves AND-wait semantics."""
    from concourse import mybir

    for f in nc.m.functions:
        for blk in f.blocks:
            insts = list(blk.instructions)
            out = []
            changed = False
            for inst in insts:
                si = inst.sync_info
                waits = list(si.on_wait) if si is not None and si.on_wait else []
                if len(waits) > 1:
                    changed = True
                    for k, w in enumerate(waits[:-1]):
                        n = mybir.InstNoOp(
                            name=f"{inst.name}-wsplit{k}", ins=[], outs=[]
                        )
                        n.engine = inst.engine
                        n.sync_info = mybir.SyncInfo(on_wait=[w], on_update=[])
                        out.append(n)
                    inst.sync_info = mybir.SyncInfo(
                        on_wait=[waits[-1]],
                        on_update=list(si.on_update) if si.on_update else [],
                    )
                out.append(inst)
            if changed:
                blk.instructions = out


def _patch_tile_drain():
    """The stock TileContext final drain carries one wait per logical proc
    (over the walrus 1-wait cap). Split them across chained single-wait nops
    on the sync queue, then run the generic multi-wait splitter over the
    whole module."""
    import concourse.tile as tile
    from concourse import mybir
    from concourse.vector_clock import ScopedClock

    if getattr(tile.TileContext, "_drain_split_patched", False):
        return

    def _drain_and_barrier(self, tick_clock, wait_clock):
        nc = self.nc
        probe = nc.sync.nop(nofuse=True)
        wait_clock.add_sem_waits(
            probe.ins, ScopedClock({None: tick_clock.global_clock})
        )
        si = probe.ins.sync_info
        waits = list(si.on_wait) if si is not None else []
        probe.ins.sync_info = mybir.SyncInfo(
            on_wait=waits[:1], on_update=[]
        )
        for w in waits[1:]:
            n = nc.sync.nop(nofuse=True)
            n.ins.sync_info = mybir.SyncInfo(on_wait=[w], on_update=[])
        nc.sync.drain()
        nc.all_engine_barrier()
        assert self.sems is not None
        popped = nc._tile_sem_poison_stack.pop()
        assert popped is self._sem_poison
        nc.clear_and_free_semaphores(list(self.sems.allocated().values()))
        nc.all_engine_barrier()
        _split_multi_waits(nc)

    tile.TileContext._drain_and_barrier = _drain_and_barrier
    tile.TileContext._drain_split_patched = True


def _patch_ldw_opt():
    """The default walrus invocation passes --enable-ldw-opt=false; enabling
    it lets walrus dedup back-to-back LDWEIGHTS with identical weights (our
    A/B moving-tile pairs reuse each stationary tile)."""
    import concourse.bass_utils as bu

    if getattr(bu, "_ldw_opt_patched", False):
        return
    orig = bu.bir_verify_and_optimise

    def patched(*args, **kwargs):
        import unittest.mock as um
        real_run = bu.run_command

        def run_with_flag(cmd, **kw):
            cmd = [c.replace("--enable-ldw-opt=false", "--enable-ldw-opt=true")
                   if isinstance(c, str) else c for c in cmd]
            return real_run(cmd, **kw)

        with um.patch.object(bu, "run_command", run_with_flag):
            return orig(*args, **kwargs)

    bu.bir_verify_and_optimise = patched
    # bass2jax imports the symbol lazily via concourse.bass_utils, but check:
    import concourse.bass2jax as b2j
    if hasattr(b2j, "bir_verify_and_optimise"):
        b2j.bir_verify_and_optimise = patched
    bu._ldw_opt_patched = True


def _build_nc():
    import concourse.bass as bass
    import concourse.tile as tile
    from concourse import mybir

    _patch_tile_drain()
    if MM_DTYPE != "f16":
        _patch_ldw_opt()

    f32 = mybir.dt.float32
    f32r = mybir.dt.float32r
    f16 = mybir.dt.float16
    mmdt = f16 if MM_DTYPE == "f16" else f32r
    Tanh = mybir.ActivationFunctionType.Tanh
    Exp = mybir.ActivationFunctionType.Exp
    AX = mybir.AxisListType.X

    nc = bass.Bass("TRN2", target_bir_lowering=False, debug=False)
    # cache-busting marker: walrus flags aren't in the NEFF cache key
    nc.sync.nop(hint="ldwopt-v1", nofuse=True)

    enc_dt = f32r if MM_DTYPE == "f32r" else f32
    enc_t = nc.dram_tensor("enc_t", [BPC, ENC_FEAT, SRC_LEN], enc_dt,
                           kind="ExternalInput").ap()
    w_eT = nc.dram_tensor("w_eT", [ENC_FEAT, DEC_HID], enc_dt,
                          kind="ExternalInput").ap()
    wd_n = nc.dram_tensor("wd_n", [DEC_HID, DEC_HID], f32,
                          kind="ExternalInput").ap()
    dec_n = nc.dram_tensor("dec_n", [BPC, DEC_HID], f32,
                           kind="ExternalInput").ap()
    b_col = nc.dram_tensor("b_col", [P, DC], f32, kind="ExternalInput").ap()
    wc_col = nc.dram_tensor("wc_col", [P, DC], enc_dt, kind="ExternalInput").ap()
    probs = nc.dram_tensor("probs", [BPC, SRC_LEN], f32,
                           kind="ExternalOutput").ap()

    with tile.TileContext(nc) as tc:
        with (
            tc.tile_pool(name="wpool", bufs=1) as wpool,
            tc.tile_pool(name="stgp", bufs=3) as stgp,
            tc.tile_pool(name="wdp", bufs=8) as wdp,
            tc.tile_pool(name="dbcp", bufs=4) as dbcp,
            tc.tile_pool(name="encp", bufs=2) as encp,
            tc.tile_pool(name="actp", bufs=8) as actp,
            tc.tile_pool(name="scp", bufs=2) as scp,
            tc.tile_pool(name="smp", bufs=1) as smp,
            tc.tile_pool(name="pse", bufs=6, space="PSUM") as pse,
            tc.tile_pool(name="pss", bufs=2, space="PSUM") as pss,
        ):
            cvt = MM_DTYPE == "f16"

            # ---- small startup DMAs; dec rows broadcast across partitions
            # for the DVE-side dec_proj ----
            b_sb = wpool.tile([P, DC], f32, tag="bcol")
            nc.sync.dma_start(b_sb[:], b_col[:, :])
            dbc = []
            for b in range(BPC):
                t = dbcp.tile([P, DEC_HID], f32, tag="dbc", name=f"dbc{b}")
                nc.sync.dma_start(t[:], dec_n[b:b + 1, :].broadcast_to([P, DEC_HID]))
                dbc.append(t)

            wc_sb = wpool.tile([P, DC], mmdt, tag="wccol")
            if cvt:
                wc_stg = wpool.tile([P, DC], f32, tag="wcstg")
                nc.sync.dma_start(wc_stg[:], wc_col[:, :])
                nc.vector.tensor_copy(wc_sb[:], wc_stg[:])
            else:
                nc.sync.dma_start(wc_sb[:], wc_col[:, :])

            we_sb = wpool.tile([P, EC * DEC_HID], mmdt, tag="we")

            def load_we(ec):
                dst = we_sb[:, ec * DEC_HID:(ec + 1) * DEC_HID]
                src = w_eT[ec * P:(ec + 1) * P, :]
                if cvt:
                    stg = stgp.tile([P, DEC_HID], f32, tag="stg")
                    nc.sync.dma_start(stg[:], src)
                    nc.gpsimd.tensor_copy(dst, stg[:])
                else:
                    nc.sync.dma_start(dst, src)

            def load_enc_chunk(t, b, h, ec):
                base = 2 * h * NTILE
                src = enc_t[b].rearrange("(c p) s -> p c s", p=P)
                if cvt:
                    stg = stgp.tile([P, 2 * NTILE], f32, tag="stg")
                    nc.sync.dma_start(stg[:], src[:, ec, base:base + 2 * NTILE])
                    nc.gpsimd.tensor_copy(t[:, ec, :], stg[:])
                else:
                    nc.sync.dma_start(
                        t[:, ec, :], src[:, ec, base:base + 2 * NTILE]
                    )

            def load_enc(b, h):
                t = encp.tile([P, EC, 2 * NTILE], mmdt, tag="enc")
                for ec in range(EC):
                    load_enc_chunk(t, b, h, ec)
                return t

            # first tile: interleave we/enc chunk DMAs so energy matmuls can
            # begin as soon as the first (we, enc) chunk pair lands
            enc_first = encp.tile([P, EC, 2 * NTILE], mmdt, tag="enc")
            for ec in range(EC):
                load_we(ec)
                load_enc_chunk(enc_first, 0, 0, ec)

            # ---- dec_proj + b_attn on DVE: bias[d, b] = b_attn[d] +
            # sum_j W_d[d, j] * dec[b, j], one fused mult+reduce per (dc, b).
            # Runs entirely off the PE; wd chunks stream at low DMA priority.
            bias_sb = wpool.tile([P, DC * BPC], f32, tag="bias")
            junk = wpool.tile([P, DEC_HID], f32, tag="ttrjunk")
            psum_dp = wpool.tile([P, 1], f32, tag="dpsum")
            wdcs = []
            for dc in range(DC):
                wdc = wdp.tile([P, DEC_HID], f32, tag="wdc", name=f"wdc{dc}")
                nc.sync.dma_start(wdc[:], wd_n[dc * P:(dc + 1) * P, :])
                wdcs.append(wdc)
            for b in range(BPC):
                for dc in range(DC):
                    nc.vector.tensor_mul(junk[:], wdcs[dc][:], dbc[b][:])
                    nc.vector.reduce_sum(psum_dp[:], junk[:], axis=AX)
                    nc.vector.tensor_scalar_add(
                        bias_sb[:, dc * BPC + b:dc * BPC + b + 1],
                        psum_dp[:], b_sb[:, dc:dc + 1],
                    )

            # ---- PE warmup: dummy matmuls fill the HAM activity window while
            # the big startup DMAs stream, so real matmuls start at K=8/8 ----
            if N_WARMUP:
                wsrc = wpool.tile([P, NTILE], f32, tag="warm")
                nc.vector.memset(wsrc[:], 0.0)
                wps = psd.tile([P, NTILE], f32, tag="warmps")
                for _ in range(N_WARMUP):
                    nc.tensor.matmul(wps[:], lhsT=wsrc[:, 0:P], rhs=wsrc[:],
                                     start=True, stop=True)

            # ---- main loop: energy -> tanh(+bias) -> w_comb reduce ----
            # scores matmuls lag the energy groups so the PE never stalls on
            # the tanh/bias chain. The first (b, h) iteration emits its first
            # three d-chunk groups ec-major so the PE can start as soon as
            # the first (we, enc) chunk pair lands.
            NSPLIT = 3

            def emit_e_mm(ps, dc, ec, enc_tile, half, start, stop):
                w_ap = we_sb[:, ec * DEC_HID + dc * P:
                             ec * DEC_HID + (dc + 1) * P]
                nc.tensor.matmul(
                    ps[:], lhsT=w_ap,
                    rhs=enc_tile[:, ec, half * NTILE:(half + 1) * NTILE],
                    start=start, stop=stop,
                )

            def emit_tanh(ps_pair, dc, b):
                bias_ap = bias_sb[:, dc * BPC + b:dc * BPC + b + 1]
                ths = []
                for k, ps in enumerate(ps_pair):
                    th = actp.tile([P, NTILE], mmdt, tag="th",
                                   name=f"th_{dc}_{b}_{k}_{nc.next_id()}")
                    nc.scalar.activation(th[:], ps[:], Tanh, bias=bias_ap)
                    ths.append(th)
                return ths

            def emit_scores(ps_s_pair, ths, dc):
                wc_ap = wc_sb[:, dc:dc + 1]
                for ps_s, th in zip(ps_s_pair, ths):
                    nc.tensor.matmul(
                        ps_s[:], lhsT=wc_ap, rhs=th[:],
                        start=(dc == 0), stop=(dc == DC - 1),
                    )

            for b in range(BPC):
                sc_line = scp.tile([1, SRC_LEN], f32, tag="sc")
                for h in range(NHALF):
                    base = 2 * h * NTILE
                    first = b == 0 and h == 0
                    enc_tile = enc_first if first else load_enc(b, h)
                    ps_sp = (pss.tile([1, NTILE], f32, tag="ps_s", name=f"ps_sA_{b}_{h}"),
                             pss.tile([1, NTILE], f32, tag="ps_s", name=f"ps_sB_{b}_{h}"))
                    pend = []
                    if first:
                        # phase A: dc 0..NSPLIT-1 ec-major (follows DMA order)
                        psl = [(pse.tile([P, NTILE], f32, tag="ps_e", name=f"psfA{i}"),
                                pse.tile([P, NTILE], f32, tag="ps_e", name=f"psfB{i}"))
                               for i in range(NSPLIT)]
                        for ec in range(EC):
                            for dc in range(NSPLIT):
                                for half in (0, 1):
                                    emit_e_mm(psl[dc][half], dc, ec, enc_tile,
                                              half, ec == 0, ec == EC - 1)
                        for dc in range(NSPLIT):
                            pend.append((emit_tanh(psl[dc], dc, b), dc))
                        dc_rest = range(NSPLIT, DC)
                    else:
                        dc_rest = range(DC)
                    for dc in dc_rest:
                        psA = pse.tile([P, NTILE], f32, tag="ps_e")
                        psB = pse.tile([P, NTILE], f32, tag="ps_e")
                        for ec in range(EC):
                            emit_e_mm(psA, dc, ec, enc_tile, 0,
                                      ec == 0, ec == EC - 1)
                            emit_e_mm(psB, dc, ec, enc_tile, 1,
                                      ec == 0, ec == EC - 1)
                        pend.append((emit_tanh((psA, psB), dc, b), dc))
                        if len(pend) > 1:
                            ths, pdc = pend.pop(0)
                            emit_scores(ps_sp, ths, pdc)
                    for ths, pdc in pend:
                        emit_scores(ps_sp, ths, pdc)
                    nc.vector.tensor_copy(
                        sc_line[:, base:base + NTILE], ps_sp[0][:]
                    )
                    nc.vector.tensor_copy(
                        sc_line[:, base + NTILE:base + 2 * NTILE], ps_sp[1][:]
                    )

                # ---- per-batch softmax (overlaps later batches' compute) ----
                neg_mx = smp.tile([1, 1], f32, tag="negmx")
                nc.vector.reduce_max(neg_mx[:], sc_line[:], axis=AX, negate=True)
                ssum = smp.tile([1, 1], f32, tag="ssum")
                ex = smp.tile([1, SRC_LEN], f32, tag="ex")
                nc.scalar.activation(ex[:], sc_line[:], Exp,
                                     bias=neg_mx[:, 0:1], accum_out=ssum[:])
                rec = smp.tile([1, 1], f32, tag="rec")
                nc.vector.reciprocal(rec[:], ssum[:])
                nc.vector.tensor_scalar_mul(ex[:], ex[:], rec[:, 0:1])
                nc.sync.dma_start(probs[b:b + 1, :], ex[0:1, :])

    return nc


def _get_nc():
    if "nc" not in _CACHED:
        _install_ntff_hook_shim()
        _CACHED["nc"] = _build_nc()
    return _CACHED["nc"]


def _prep_in_maps(decoder_state, encoder_annotation_seq, W_attn, b_attn, w_comb):
    dec = np.asarray(decoder_state, np.float32)
    enc = np.asarray(encoder_annotation_seq, np.float32)
    W = np.asarray(W_attn, np.float32)
    ba = np.asarray(b_attn, np.float32)
    wc = np.asarray(w_comb, np.float32)

    # layout-only host prep (no FLOPs)
    encT = np.ascontiguousarray(enc.transpose(1, 2, 0))      # [bs, e, s]
    w_eT = np.ascontiguousarray(W[:, :ENC_FEAT].T)           # [e, d]
    wd_n = np.ascontiguousarray(W[:, ENC_FEAT:])             # [d, j]
    b_col = np.ascontiguousarray(ba.reshape(DC, P).T)        # [128, 8]
    wc_col = np.ascontiguousarray(wc.reshape(DC, P).T)       # [128, 8]

    in_maps = []
    for c in range(N_CORES):
        sl = slice(c * BPC, (c + 1) * BPC)
        in_maps.append({
            "enc_t": np.ascontiguousarray(encT[sl]),
            "w_eT": w_eT,
            "wd_n": wd_n,
            "dec_n": np.ascontiguousarray(dec[sl]),
            "b_col": b_col,
            "wc_col": wc_col,
        })
    return in_maps


def run(inputs: dict, trace: bool = False):
    """Run the SPMD kernel. Returns (full_output [32, 2048], BassKernelResults)."""
    from concourse.bass_utils import run_bass_kernel_spmd

    nc = _get_nc()
    in_maps = _prep_in_maps(**inputs)
    res = run_bass_kernel_spmd(
        nc, in_maps, core_ids=list(range(N_CORES)), trace=trace
    )
    out = np.concatenate(
        [res.results[c]["probs"] for c in range(N_CORES)], axis=0
    ).astype(np.float32)
    return out, res


def kernel(decoder_state, encoder_annotation_seq, W_attn, b_attn, w_comb):
    out, _ = run(dict(
        decoder_state=decoder_state,
        encoder_annotation_seq=encoder_annotation_seq,
        W_attn=W_attn,
        b_attn=b_attn,
        w_comb=w_comb,
    ))
    return out
